# revision 1
# baseline (speedup 1.0000x reference)
"""Trainium2 Bass kernel for nn_BinaryBiaffine2 (biaffine dependency scorer).

Math (per batch b):
    h_dep  = leaky_relu(hidden @ W_dep  + b_dep)             [L, 500]
    h_head = leaky_relu(hidden @ W_head + b_head)            [L, 500]
    dep    = h_dep  @ Wc[:500]                               [L, 2]
    head   = h_head @ Wc[500:]                               [L, 2]
    out[i, j, c] = dep[i, c] + head[j, c] + bc[c]            [L, L, 2]

Sharding: data-parallel over batch, 2 batches per core on 8 cores.

Per-core strategy (v2):
  - hidden loaded natural ([tok, d]) in 1MB chunks, transposed 128x128 on
    the PE (fp32 transpose mode) into hT tiles [d, tok] (f32r).
  - Both MLP branches computed in [m, tok] layout (lhsT = W slice, rhs =
    hT): psum tiles are [128, 1024] (2 banks), matmuls write 512-halves.
  - leaky(x+b) = relu(0.99x + 0.99b) + 0.01*(x+b), exactly:
    ACT Relu(scale=0.99, bias=0.99b) + DVE tensor_scalar((ps+b)*0.01) +
    DVE add; output rounds to f32r for the downstream score matmuls.
  - head scores pre-broadcast across partitions via matmul with a
    partition-replicated Wc column as stationary (+bc folded into the
    PSUM->SBUF copy).
  - dep scores via an M=2 matmul (lhsT = Wc_dep tile [m,2]) giving
    depT [2, tok], then 8 tiny PE transposes -> per-token scalars
    [128, 2] per i-tile.
  - out[i, j, c] = head_bc_c[j] + dep_c[i]: one elementwise op per
    (i-tile, c) spread across Pool/ACT/DVE, into [128, 1024, 2] tiles
    DMAed out on alternating sync/scalar HWDGE queues. The dep branch
    runs per token-half so stores start at half-batch granularity;
    next-batch hidden loads are prefetched before out-ops claim the
    Pool queue.

  CoreSim cost model: ~108.5us/core (PE busy ~79us). HW rel err vs the
  fp32 reference: ~2.6e-4 (float32r rounding).
"""

import os
import sys

if "/opt/trn_rl_repo" not in sys.path:
    sys.path.insert(0, "/opt/trn_rl_repo")

import numpy as np

B, L, D = 16, 1024, 1024
MLP = 500
MLP_PAD = 512
NEG_SLOPE = 0.01
N_CORES = 8
B_PER_CORE = B // N_CORES
P = 128
N_MT = MLP_PAD // P  # 4 m-tiles of 128
N_KO = D // P        # 8 d-slices of 128
N_TSUB = L // P      # 8 token subtiles per batch

# "f32r" (full-rate, ~12-bit-mantissa) or "f32" (exact, 4x slower on PE)
MM_DTYPE = os.environ.get("BB_MM_DTYPE", "f32r")

_CACHE = {}


def _build_nc():
    import concourse.tile as tile
    from concourse import bacc, mybir
    from concourse.bass import ts
    from contextlib import ExitStack

    f32 = mybir.dt.float32
    mm_dt = {"f32r": mybir.dt.float32r, "f32": mybir.dt.float32}[MM_DTYPE]
    Relu = mybir.ActivationFunctionType.Relu
    Identity = mybir.ActivationFunctionType.Identity
    Add = mybir.AluOpType.add
    Mult = mybir.AluOpType.mult

    nc = bacc.Bacc()

    hidden = nc.dram_tensor("hidden", [B_PER_CORE, L, D], f32, kind="ExternalInput")
    w_dep_d = nc.dram_tensor("w_dep", [D, MLP_PAD], mm_dt, kind="ExternalInput")
    w_head_d = nc.dram_tensor("w_head", [D, MLP_PAD], mm_dt, kind="ExternalInput")
    # bias tiles: columns (2*mt, 2*mt+1) = (0.99*b, b) for m-tile mt
    b_dep_d = nc.dram_tensor("b_dep_t", [P, 2 * N_MT], f32, kind="ExternalInput")
    b_head_d = nc.dram_tensor("b_head_t", [P, 2 * N_MT], f32, kind="ExternalInput")
    wc_dep_d = nc.dram_tensor("wc_dep_t", [P, N_MT, 2], mm_dt, kind="ExternalInput")
    wc_head_d = nc.dram_tensor("wc_head_bc", [P, 2, N_MT, P], mm_dt, kind="ExternalInput")
    bc_d = nc.dram_tensor("bc_bc", [P, 2], f32, kind="ExternalInput")
    ident_d = nc.dram_tensor("ident", [P, P], f32, kind="ExternalInput")
    out_d = nc.dram_tensor("out", [B_PER_CORE, L, L, 2], f32, kind="ExternalOutput")

    with tile.TileContext(nc) as tc:
        with ExitStack() as ctx:
            const = ctx.enter_context(tc.tile_pool(name="const", bufs=1))
            hnat_p = ctx.enter_context(tc.tile_pool(name="hnat", bufs=4))
            hT_p = ctx.enter_context(tc.tile_pool(name="hT", bufs=16))
            lh_p = ctx.enter_context(tc.tile_pool(name="lh", bufs=5))
            tmp_p = ctx.enter_context(tc.tile_pool(name="tmp", bufs=2))
            dept_p = ctx.enter_context(tc.tile_pool(name="dept", bufs=2))
            depsc_p = ctx.enter_context(tc.tile_pool(name="depsc", bufs=2))
            hbc_p = ctx.enter_context(tc.tile_pool(name="hbc", bufs=2))
            out_p = ctx.enter_context(tc.tile_pool(name="outp", bufs=5))
            tr_ps = ctx.enter_context(tc.tile_pool(name="trps", bufs=4, space="PSUM"))
            big_ps = ctx.enter_context(tc.tile_pool(name="bigps", bufs=2, space="PSUM"))

            # Small constants first so PE transposes (need ident) are not
            # stuck behind the 4MB of weights; weights split across the two
            # HWDGE queues (SP + ACT).
            ident_sb = const.tile([P, P], f32)
            nc.sync.dma_start(ident_sb, ident_d[:, :])
            b_sb = {}
            b_dep_sb = const.tile([P, 2 * N_MT], f32)
            nc.sync.dma_start(b_dep_sb, b_dep_d[:, :])
            b_head_sb = const.tile([P, 2 * N_MT], f32)
            nc.sync.dma_start(b_head_sb, b_head_d[:, :])
            b_sb["dep"], b_sb["head"] = b_dep_sb, b_head_sb
            wc_dep_sb = const.tile([P, N_MT, 2], mm_dt)
            nc.sync.dma_start(wc_dep_sb, wc_dep_d[:, :, :])
            bc_sb = const.tile([P, 2], f32)
            nc.sync.dma_start(bc_sb, bc_d[:, :])
            wc_head_sb = const.tile([P, 2, N_MT, P], mm_dt)
            nc.sync.dma_start(wc_head_sb, wc_head_d[:, :, :, :])
            w_sb = {}
            w_head_sb = const.tile([P, N_KO, MLP_PAD], mm_dt)
            nc.scalar.dma_start(w_head_sb, w_head_d[:, :].rearrange("(ko p) m -> p ko m", p=P))
            w_dep_sb = const.tile([P, N_KO, MLP_PAD], mm_dt)
            nc.sync.dma_start(w_dep_sb, w_dep_d[:, :].rearrange("(ko p) m -> p ko m", p=P))
            w_sb["dep"], w_sb["head"] = w_dep_sb, w_head_sb

            out_uc = [0]  # rotating engine assignment for output units

            def load_batch(b):
                h_nats = []
                hid_r = hidden[:, :, :]
                for tp in range(N_TSUB // 2):
                    h_nat = hnat_p.tile([P, 2, D], f32, name="h_nat")
                    if b == 0:
                        # finer chunks so the first PE transposes start sooner
                        for s in range(2):
                            nc.gpsimd.dma_start(
                                h_nat[:, s],
                                hid_r[b, ts(2 * tp + s, P), :],
                            )
                    else:
                        nc.gpsimd.dma_start(
                            h_nat,
                            hid_r[b, ts(tp, 2 * P), :].rearrange("(s p) d -> p s d", p=P),
                        )
                    h_nats.append(h_nat)
                return h_nats

            loaded = load_batch(0)
            for b in range(B_PER_CORE):
                h_nats = loaded

                # ---- PE-transpose into hT tiles [d=128, tok=512] ----
                hTs = {}
                for half in range(2):
                    for ko in range(N_KO):
                        ptr = tr_ps.tile([P, 512], f32, name="ptr")
                        for q in range(4):
                            tsub = half * 4 + q
                            nc.tensor.matmul(
                                ptr[:, ts(q, P)],
                                lhsT=h_nats[tsub // 2][:, tsub % 2, ts(ko, P)],
                                rhs=ident_sb,
                                is_transpose=True,
                                start=True,
                                stop=True,
                            )
                        hT = hT_p.tile([P, 512], mm_dt, name="hT")
                        nc.vector.tensor_copy(hT, ptr)
                        hTs[half, ko] = hT

                # prefetch next batch on the Pool queue before out-ops claim it
                if b + 1 < B_PER_CORE:
                    loaded = load_batch(b + 1)

                # ---- branches in [m, tok] layout, scores right after ----
                def branch_mlp(br):
                    tiles = {}
                    for mt in range(N_MT):
                        ps = big_ps.tile([P, 2 * 512], f32, name="ps")
                        for half in range(2):
                            for ko in range(N_KO):
                                nc.tensor.matmul(
                                    ps[:, ts(half, 512)],
                                    lhsT=w_sb[br][:, ko, ts(mt, P)],
                                    rhs=hTs[half, ko],
                                    start=(ko == 0),
                                    stop=(ko == N_KO - 1),
                                )
                        lh = lh_p.tile([P, 2 * 512], mm_dt, name="lh")
                        lt = tmp_p.tile([P, 2 * 512], f32, name="lt")
                        nc.scalar.activation(
                            lh, ps, Relu,
                            bias=b_sb[br][:, 2 * mt : 2 * mt + 1],
                            scale=1.0 - NEG_SLOPE,
                        )
                        nc.vector.tensor_scalar(
                            lt, ps,
                            b_sb[br][:, 2 * mt + 1 : 2 * mt + 2], NEG_SLOPE,
                            Add, Mult,
                        )
                        nc.vector.tensor_add(lh, lh, lt)
                        tiles[mt] = lh
                    return tiles

                lh_head = branch_mlp("head")

                # ---- head scores, partition-broadcast, +bc folded ----
                head_bcs = {}
                for c in range(2):
                    pbc = big_ps.tile([P, 2 * 512], f32, name="ps")
                    for half in range(2):
                        for mt in range(N_MT):
                            nc.tensor.matmul(
                                pbc[:, ts(half, 512)],
                                lhsT=wc_head_sb[:, c, mt, :],
                                rhs=lh_head[mt][:, ts(half, 512)],
                                start=(mt == 0),
                                stop=(mt == N_MT - 1),
                            )
                    hb = hbc_p.tile([P, L], f32, name="hb")
                    nc.scalar.activation(hb, pbc, Identity, bias=bc_sb[:, c : c + 1])
                    head_bcs[c] = hb

                # ---- dep branch per token-half so output/stores start at
                # half-batch granularity (hides the store-bandwidth tail).
                # Both halves' matmuls+leaky are emitted before either half's
                # score chain so half-B's PE work fills half-A's ACT/DVE
                # latency. ----
                dep_all = depsc_p.tile([P, 2 * N_TSUB], f32, name="dep_all")
                pds = tr_ps.tile([P, 2 * N_TSUB], f32, name="ptr", padded_shape=[P, 512])
                lh_dep = {}
                for half in range(2):
                    for mt in range(N_MT):
                        psd = tr_ps.tile([P, 512], f32, name="ptr")
                        for ko in range(N_KO):
                            nc.tensor.matmul(
                                psd,
                                lhsT=w_sb["dep"][:, ko, ts(mt, P)],
                                rhs=hTs[half, ko],
                                start=(ko == 0),
                                stop=(ko == N_KO - 1),
                            )
                        lh = lh_p.tile([P, 512], mm_dt, name="lhd", bufs=8)
                        lt = tmp_p.tile([P, 512], f32, name="ltd", bufs=4)
                        nc.scalar.activation(
                            lh, psd, Relu,
                            bias=b_sb["dep"][:, 2 * mt : 2 * mt + 1],
                            scale=1.0 - NEG_SLOPE,
                        )
                        nc.vector.tensor_scalar(
                            lt, psd,
                            b_sb["dep"][:, 2 * mt + 1 : 2 * mt + 2], NEG_SLOPE,
                            Add, Mult,
                        )
                        nc.vector.tensor_add(lh, lh, lt)
                        lh_dep[half, mt] = lh

                # both halves' score chains first (keeps the ACT queue clear
                # of out-ops so half-1's chain is not delayed)
                for half in range(2):
                    # dep scores: M=2 matmul -> depT [2, 512]
                    dep_t = dept_p.tile([2, 512], f32, name="dep_t")
                    pdt = tr_ps.tile([2, 512], f32, name="ptr", padded_shape=[P, 512])
                    for mt in range(N_MT):
                        nc.tensor.matmul(
                            pdt,
                            lhsT=wc_dep_sb[:, mt, :],
                            rhs=lh_dep[half, mt],
                            start=(mt == 0),
                            stop=(mt == N_MT - 1),
                        )
                    nc.scalar.activation(dep_t, pdt, Identity)

                    # transpose to per-token scalars [128, 2] per i-tile
                    for q in range(4):
                        tsub = half * 4 + q
                        nc.tensor.matmul(
                            pds[:, 2 * tsub : 2 * tsub + 2],
                            lhsT=dep_t[:, ts(q, P)],
                            rhs=ident_sb[:2, :2],
                            is_transpose=True,
                            start=True,
                            stop=True,
                        )
                    nc.vector.tensor_copy(
                        dep_all[:, 8 * half : 8 * half + 8],
                        pds[:, 8 * half : 8 * half + 8],
                    )

                # pairwise add + store; the very last half spreads its ops
                # evenly over Pool/ACT/DVE to compress the exposed tail
                last_half = b == B_PER_CORE - 1
                for tsub in range(N_TSUB):
                    ot = out_p.tile([P, L, 2], f32, name="ot")
                    d0 = dep_all[:, 2 * tsub : 2 * tsub + 1]
                    d1 = dep_all[:, 2 * tsub + 1 : 2 * tsub + 2]
                    if last_half and tsub >= 4:
                        pick = [(0, 1), (2, 0), (1, 2), (0, 2)][tsub - 4]
                    else:
                        pick = (0, 1) if tsub % 2 == 0 else (0, 2)
                    for sel, (dst, src, dap) in zip(
                        pick, [(ot[:, :, 0], head_bcs[0], d0), (ot[:, :, 1], head_bcs[1], d1)]
                    ):
                        if sel == 0:
                            nc.gpsimd.tensor_scalar(dst, src, dap, None, Add)
                        elif sel == 1:
                            nc.scalar.activation(dst, src, Identity, bias=dap)
                        else:
                            nc.vector.tensor_scalar(dst, src, dap, None, Add)
                    eng = nc.sync if tsub % 2 == 0 else nc.scalar
                    eng.dma_start(out_d[b, ts(tsub, P)], ot)

    nc.compile()
    return nc


def _prep_consts(W_dep, b_dep, W_head, b_head, Wc, bc):
    f = np.float32

    def pad_w(W):
        Wp = np.zeros((D, MLP_PAD), f)
        Wp[:, :MLP] = W
        return Wp

    def bias_t(bvec):
        bp = np.zeros((MLP_PAD,), f)
        bp[:MLP] = bvec
        bt = bp.reshape(N_MT, P).T  # [P, N_MT]
        out = np.empty((P, 2 * N_MT), f)
        out[:, 0::2] = (1.0 - NEG_SLOPE) * bt
        out[:, 1::2] = bt
        return out

    wc_dep_pad = np.zeros((MLP_PAD, 2), f)
    wc_dep_pad[:MLP] = Wc[:MLP]
    wc_dep_t = wc_dep_pad.reshape(N_MT, P, 2).transpose(1, 0, 2).copy()  # [P,mt,2]

    wc_head_pad = np.zeros((MLP_PAD, 2), f)
    wc_head_pad[:MLP] = Wc[MLP:]
    wh = wc_head_pad.reshape(N_MT, P, 2).transpose(1, 2, 0)  # [P, 2, N_MT]
    wc_head_bc = np.broadcast_to(wh[:, :, :, None], (P, 2, N_MT, P)).copy()

    return {
        "w_dep": pad_w(W_dep),
        "w_head": pad_w(W_head),
        "b_dep_t": bias_t(b_dep),
        "b_head_t": bias_t(b_head),
        "wc_dep_t": wc_dep_t,
        "wc_head_bc": wc_head_bc,
        "bc_bc": np.broadcast_to(bc.astype(f), (P, 2)).copy(),
        "ident": np.eye(P, dtype=f),
    }


def kernel(hidden_state, W_dep, b_dep, W_head, b_head, Wc, bc):
    from concourse.bass_utils import run_bass_kernel_spmd

    hidden_state = np.ascontiguousarray(np.asarray(hidden_state, dtype=np.float32))
    consts = _prep_consts(
        np.asarray(W_dep, np.float32),
        np.asarray(b_dep, np.float32),
        np.asarray(W_head, np.float32),
        np.asarray(b_head, np.float32),
        np.asarray(Wc, np.float32),
        np.asarray(bc, np.float32),
    )

    if "nc" not in _CACHE:
        _CACHE["nc"] = _build_nc()
    nc = _CACHE["nc"]

    in_maps = []
    for k in range(N_CORES):
        m = {"hidden": hidden_state[k * B_PER_CORE : (k + 1) * B_PER_CORE]}
        m.update(consts)
        in_maps.append(m)

    trace = bool(int(os.environ.get("BB_TRACE", "0")))
    if not trace:
        # The NTFF profiling hook (antenv.axon_hooks) is absent in this
        # container; a stray BASS_TRACE=1 would crash the run. Force off.
        os.environ["BASS_NEVER_TRACE"] = "1"
    res = run_bass_kernel_spmd(nc, in_maps, list(range(N_CORES)), trace=trace)
    _CACHE["last_results"] = res
    out = np.concatenate([res.results[k]["out"] for k in range(N_CORES)], axis=0)
    return out



# revision 61
# speedup vs baseline: 1.4138x; 1.4138x over previous
"""Trainium2 Bass kernel for nn_BinaryBiaffine2 (biaffine dependency scorer).

Math (per batch b):
    h_dep  = leaky_relu(hidden @ W_dep  + b_dep)             [L, 500]
    h_head = leaky_relu(hidden @ W_head + b_head)            [L, 500]
    dep    = h_dep  @ Wc[:500]                               [L, 2]
    head   = h_head @ Wc[500:]                               [L, 2]
    out[i, j, c] = dep[i, c] + head[j, c] + bc[c]            [L, L, 2]

Sharding: data-parallel over batch, 2 batches per core on 8 cores.

Per-core strategy (v3):
  - hidden is pre-transposed on the HOST to [b, d, tok]; DMAed straight
    into [d, tok] SBUF tiles (f32r reinterpret) — no PE transposes, no
    PSUM->SBUF copies for the activations.
  - MLP in [m, tok] layout with W as the stationary operand (lhsT =
    W[d-slice, m-tile], rhs = hT[d-slice, tok-half]); PSUM [128, 1024]
    tiles, matmuls per 512-column bank half, 8-step k accumulation.
  - leaky_relu in ONE ACT op per m-tile: parametric_relu (alpha=0.01)
    with the bias folded in; output rounds to f32r for the score matmuls.
  - head scores [2, tok] via lhsT=Wc_head[m-slice, 2] accumulation, +bc
    on the PSUM->SBUF copy; broadcast across partitions with a k=1
    ones-matmul (2 x 512-col per channel).
  - dep scores computed directly per token tile: tiny [128, 2] matmuls
    (lhsT = lh_dep[m-tile, tok-slice], rhs = Wc_dep[m-slice, 2]) into a
    shared PSUM bank, then a tiny DVE copy -> per-partition scalars.
  - out[i, j, c] = hb_c[j] + dep_c[i]: one elementwise op per (i-tile, c)
    spread across ACT/DVE/Pool, DMAed out round-robin over the three
    queues (SP / ACT HWDGE + Pool SWDGE, which run concurrently).
  - cross-batch software pipelining: batch b's second-half dep scores and
    out-ops are emitted inside batch b+1's first head m-tile so the PE
    never waits on the leaky chase.
"""

import os
import sys

if "/opt/trn_rl_repo" not in sys.path:
    sys.path.insert(0, "/opt/trn_rl_repo")

import numpy as np

B, L, D = 16, 1024, 1024
MLP = 500
MLP_PAD = 512
NEG_SLOPE = 0.01
N_CORES = 8
B_PER_CORE = B // N_CORES
P = 128
N_MT = MLP_PAD // P  # 4 m-tiles of 128
N_KO = D // P        # 8 d-slices of 128
N_TSUB = L // P      # 8 token subtiles per batch
HB = 512             # psum bank width in f32
# batch-0 hidden DMA arrival order across the 3 queues (sync: 0,3,6 /
# gpsimd: 2,5 / scalar: 1,4,7 after the first weight chunk)
KO_ARRIVAL = [0, 1, 3, 2, 4, 5, 7, 6]

# "f32r" (full-rate, reduced-mantissa) or "f32" (exact, 4x slower on PE)
MM_DTYPE = os.environ.get("BB_MM_DTYPE", "f32r")
# "prelu": one ACT op; "decomp": relu(y) - 0.01*relu(-y) (2 ACT + 1 DVE)
LEAKY = os.environ.get("BB_LEAKY", "prelu")

_CACHE = {}


def _build_nc():
    import concourse.tile as tile
    from concourse import bacc, mybir
    from concourse.bass import ts
    from contextlib import ExitStack

    f32 = mybir.dt.float32
    mm_dt = {"f32r": mybir.dt.float32r, "f32": mybir.dt.float32}[MM_DTYPE]
    Prelu = mybir.ActivationFunctionType.Prelu
    Relu = mybir.ActivationFunctionType.Relu
    Identity = mybir.ActivationFunctionType.Identity
    Add = mybir.AluOpType.add

    nc = bacc.Bacc()

    # hidden arrives pre-transposed: [b, d, tok]
    hid_d = nc.dram_tensor("hidden_t", [B_PER_CORE, D, L], mm_dt, kind="ExternalInput")
    # weights host-arranged [p(d within slice), mt, ko, m-within-tile]
    w_dep_d = nc.dram_tensor("w_dep_r", [P, N_MT, N_KO, P], mm_dt, kind="ExternalInput")
    w_head_d = nc.dram_tensor("w_head_r", [P, N_MT, N_KO, P], mm_dt, kind="ExternalInput")
    b_dep_d = nc.dram_tensor("b_dep_t", [P, 2 * N_MT], f32, kind="ExternalInput")
    b_head_d = nc.dram_tensor("b_head_t", [P, 2 * N_MT], f32, kind="ExternalInput")
    wc_dep_d = nc.dram_tensor("wc_dep_t", [P, N_MT, 2], mm_dt, kind="ExternalInput")
    wc_head_d = nc.dram_tensor("wc_head_t", [P, N_MT, 2], mm_dt, kind="ExternalInput")
    sel_d = nc.dram_tensor("sel_r", [2, 2, P], mm_dt, kind="ExternalInput")
    bc_d = nc.dram_tensor("bc_2", [2, 1], f32, kind="ExternalInput")
    out_d = nc.dram_tensor("out", [B_PER_CORE, L, L, 2], f32, kind="ExternalOutput")

    with tile.TileContext(nc) as tc:
        with ExitStack() as ctx:
            const = ctx.enter_context(tc.tile_pool(name="const", bufs=1))
            hT_p = ctx.enter_context(tc.tile_pool(name="hT", bufs=2 * N_KO))
            lhh_p = ctx.enter_context(tc.tile_pool(name="lhh", bufs=N_MT))
            lhd_p = ctx.enter_context(tc.tile_pool(name="lhd", bufs=N_MT))
            lhdh_p = ctx.enter_context(tc.tile_pool(name="lhdh", bufs=2 * N_MT))
            hsb_p = ctx.enter_context(tc.tile_pool(name="hsb", bufs=2))
            hb_p = ctx.enter_context(tc.tile_pool(name="hb", bufs=3))
            out_p = ctx.enter_context(
                tc.tile_pool(name="outp", bufs=5 if LEAKY == "prelu" else 4))
            big_ps = ctx.enter_context(tc.tile_pool(name="bigps", bufs=3, space="PSUM"))
            depall_p = ctx.enter_context(tc.tile_pool(name="depall", bufs=3))
            dsc_ps = ctx.enter_context(tc.tile_pool(name="dscps", bufs=1, space="PSUM"))

            # --- constants ---------------------------------------------------
            # warmup operand via memset (no DMA latency): its matmuls start
            # the PE p-state ramp clock while hidden/weights stream in
            wu_sb = const.tile([P, 2], f32)
            nc.gpsimd.memset(wu_sb, 0.0)
            b_head_sb = const.tile([P, 2 * N_MT], f32)
            b_dep_sb = const.tile([P, 2 * N_MT], f32)
            if LEAKY != "prelu":
                lt_p = ctx.enter_context(tc.tile_pool(name="ltp", bufs=2))
            b_sb = {"dep": b_dep_sb, "head": b_head_sb}

            # batch-0 hidden split across all three queues. The scalar (ACT)
            # queue must fully drain before ACT's first Prelu (~7.5us): a DMA
            # on a queue blocks that engine's later compute for the whole
            # transfer in this machine model. So scalar gets only the first
            # head-weight chunk + 3 hidden tiles; everything else goes on
            # sync/gpsimd ordered by first use.
            w_head_sb = const.tile([P, N_MT, N_KO, P], mm_dt)
            w_dep_sb = const.tile([P, N_MT, N_KO, P], mm_dt)
            w_sb = {"dep": w_dep_sb, "head": w_head_sb}

            hts0 = [hT_p.tile([P, L], mm_dt, name="hT") for _ in range(N_KO)]
            nc.scalar.dma_start(w_head_sb[:, 0], w_head_d[:, 0])
            nc.gpsimd.dma_start(w_head_sb[:, 1], w_head_d[:, 1])
            nc.gpsimd.dma_start(b_head_sb, b_head_d[:, :])
            nc.gpsimd.dma_start(b_dep_sb, b_dep_d[:, :])
            for ko in (0, 3, 2, 6):
                nc.sync.dma_start(hts0[ko], hid_d[0, ts(ko, P), :])
            for ko in (1, 4, 7):
                nc.scalar.dma_start(hts0[ko], hid_d[0, ts(ko, P), :])
            # rest of the weights: head mt2/mt3 + dep mt0/mt1 on sync (ahead
            # of the b1 prefetch), dep mt2/mt3 on the Pool queue
            nc.sync.dma_start(w_head_sb[:, 2], w_head_d[:, 2])
            nc.sync.dma_start(w_head_sb[:, 3], w_head_d[:, 3])
            nc.sync.dma_start(w_dep_sb[:, 0], w_dep_d[:, 0])
            nc.sync.dma_start(w_dep_sb[:, 1], w_dep_d[:, 1])

            # remaining consts on the Pool queue, by first use
            nc.gpsimd.dma_start(hts0[5], hid_d[0, ts(5, P), :])
            wc_head_sb = const.tile([P, N_MT, 2], mm_dt)
            nc.gpsimd.dma_start(wc_head_sb, wc_head_d[:, :, :])
            bc_sb = const.tile([2, 1], f32)
            nc.gpsimd.dma_start(bc_sb, bc_d[:, :])
            sel_sb = const.tile([2, 2, P], mm_dt)
            nc.gpsimd.dma_start(sel_sb, sel_d[:, :, :])
            wc_dep_sb = const.tile([P, N_MT, 2], mm_dt)
            nc.gpsimd.dma_start(wc_dep_sb, wc_dep_d[:, :, :])
            nc.gpsimd.dma_start(w_dep_sb[:, 2], w_dep_d[:, 2])
            nc.gpsimd.dma_start(w_dep_sb[:, 3], w_dep_d[:, 3])

            # PE warmup: tiny matmuls so the p-state ramp (full speed after
            # 3us of busy) completes during the initial DMA fill
            wu_ps = dsc_ps.tile([2, 2], f32, name="dp", padded_shape=[P, HB])
            for _ in range(30):
                nc.tensor.matmul(wu_ps, lhsT=wu_sb, rhs=wu_sb, start=True, stop=True)

            def load_hidden(b):
                hts = []
                for ko in range(N_KO):
                    t = hT_p.tile([P, L], mm_dt, name="hT")
                    nc.sync.dma_start(t, hid_d[b, ts(ko, P), :])
                    hts.append(t)
                return hts

            # --- per-batch stages -------------------------------------------
            def emit_leaky(lh_ap, ps_ap, br, mt):
                if LEAKY == "prelu":
                    nc.scalar.activation(
                        lh_ap, ps_ap, Prelu,
                        bias=b_sb[br][:, mt : mt + 1],
                        alpha=NEG_SLOPE,
                    )
                else:
                    # exact: leaky(y) = relu(y) - 0.01*relu(-y), y = x + b
                    r2 = lt_p.tile([P, L], f32, name="lt")
                    r2_ap = r2[:, : lh_ap.shape[-1]] if lh_ap.shape[-1] != L else r2
                    nc.scalar.activation(
                        lh_ap, ps_ap, Relu,
                        bias=b_sb[br][:, mt : mt + 1],
                    )
                    nc.scalar.activation(
                        r2_ap, ps_ap, Relu,
                        bias=b_sb[br][:, N_MT + mt : N_MT + mt + 1],
                        scale=-NEG_SLOPE,
                    )
                    nc.vector.tensor_sub(lh_ap, lh_ap, r2_ap)

            def leaky(br, mt, ps):
                pool = lhh_p if br == "head" else lhd_p
                lh = pool.tile([P, L], mm_dt, name="lh_" + br)
                emit_leaky(lh, ps, br, mt)
                return lh

            def mlp_mtile(br, mt, hts):
                ps = big_ps.tile([P, L], f32, name="ps")
                for half in range(2):
                    for ko in range(N_KO):
                        nc.tensor.matmul(
                            ps[:, ts(half, HB)],
                            lhsT=w_sb[br][:, mt, ko, :],
                            rhs=hts[ko][:, ts(half, HB)],
                            start=(ko == 0),
                            stop=(ko == N_KO - 1),
                        )
                return leaky(br, mt, ps)

            def mlp_mtile_seg(br, mt, hts, t0, ntsub):
                """Partial-token-width dep m-tile (last batch): lets earlier
                segments' outputs store while later ones still compute.
                ntsub*128 must stay >= 256 for full-rate f32r."""
                w = ntsub * P
                ps = big_ps.tile([P, L], f32, name="ps")
                for ko in range(N_KO):
                    nc.tensor.matmul(
                        ps[:, :w],
                        lhsT=w_sb[br][:, mt, ko, :],
                        rhs=hts[ko][:, t0 * P : t0 * P + w],
                        start=(ko == 0),
                        stop=(ko == N_KO - 1),
                    )
                lh = lhdh_p.tile([P, HB], mm_dt, name="lh_h")
                emit_leaky(lh[:, :w], ps[:, :w], br, mt)
                return lh

            def mlp_mtile_pair_komajor(br, mts, hts, ko_order):
                """Batch-0 fill: two m-tiles interleaved ko-major in DMA
                arrival order, so the PE chews each hidden tile as it lands."""
                pss = {mt: big_ps.tile([P, L], f32, name="ps") for mt in mts}
                for i, ko in enumerate(ko_order):
                    for mt in mts:
                        for half in range(2):
                            nc.tensor.matmul(
                                pss[mt][:, ts(half, HB)],
                                lhsT=w_sb[br][:, mt, ko, :],
                                rhs=hts[ko][:, ts(half, HB)],
                                start=(i == 0),
                                stop=(i == N_KO - 1),
                            )
                return [leaky(br, mt, pss[mt]) for mt in mts]

            def head_scores(lh_head):
                hs_ps = big_ps.tile([2, L], f32, name="ps", padded_shape=[P, L])
                for half in range(2):
                    for mt in range(N_MT):
                        nc.tensor.matmul(
                            hs_ps[:, ts(half, HB)],
                            lhsT=wc_head_sb[:, mt, :],
                            rhs=lh_head[mt][:, ts(half, HB)],
                            start=(mt == 0),
                            stop=(mt == N_MT - 1),
                        )
                head_sb = hsb_p.tile([2, L], mm_dt, name="head_sb")
                nc.scalar.activation(head_sb, hs_ps, Identity, bias=bc_sb[:, :])
                return head_sb

            def bcast(head_sb, c):
                hb_ps = big_ps.tile([P, L], f32, name="ps")
                for half in range(2):
                    nc.tensor.matmul(
                        hb_ps[:, ts(half, HB)],
                        lhsT=sel_sb[:, c, :],
                        rhs=head_sb[:, ts(half, HB)],
                        start=True,
                        stop=True,
                    )
                hb = hb_p.tile([P, L], f32, name="hb")
                nc.scalar.activation(hb, hb_ps, Identity)
                return hb

            def dsc_open():
                # [P, mt, q, c]: each (mt, q) matmul is self-contained
                # (start+stop) -- only one PSUM accumulation group may be
                # pending per zero region, so no cross-mt accumulation here
                return dsc_ps.tile([P, 2 * N_MT * (N_TSUB // 2)], f32,
                                   name="dp", padded_shape=[P, HB])

            def dsc_mt(dp, lh_mt, mt, qs):
                """qs: list of (psum q idx, lh col tsub idx)."""
                base = 2 * (N_TSUB // 2) * mt
                for qd, ql in qs:
                    nc.tensor.matmul(
                        dp[:, base + 2 * qd : base + 2 * qd + 2],
                        lhsT=lh_mt[:, ts(ql, P)],
                        rhs=wc_dep_sb[:, mt, :],
                        start=True,
                        stop=True,
                    )

            def dsc_close(dp, n):
                dep_all = depall_p.tile([P, 2 * N_TSUB // 2], f32, name="dep_all")
                w = 2 * (N_TSUB // 2)
                da = dep_all[:, : 2 * n]
                nc.vector.tensor_copy(da, dp[:, 0 : 2 * n])
                nc.vector.tensor_add(da, da, dp[:, w : w + 2 * n])
                nc.vector.tensor_add(da, da, dp[:, 2 * w : 2 * w + 2 * n])
                nc.vector.tensor_add(da, da, dp[:, 3 * w : 3 * w + 2 * n])
                return dep_all

            def dep_scores(lh_dep, half):
                dp = dsc_open()
                qs = [(q, half * (N_TSUB // 2) + q) for q in range(N_TSUB // 2)]
                for mt in range(N_MT):
                    dsc_mt(dp, lh_dep[mt], mt, qs)
                return dsc_close(dp, N_TSUB // 2)

            def out_tiles(b, tsubs, dep_all, hbs, engs, queues,
                          ops_first=False):
                """engs: per-op engine cycle; queues: per-tile store queue or
                (qa, qb) pair for a split half-tile store. ops_first: emit
                every op before any store so an engine's own store DMA never
                delays its remaining ops (tail mode)."""
                stores = []
                for q, tsub in enumerate(tsubs):
                    ot = out_p.tile([P, L, 2], f32, name="ot")
                    for c in range(2):
                        eng = engs[(2 * q + c) % len(engs)]
                        dap = dep_all[:, 2 * q + c : 2 * q + c + 1]
                        if eng is nc.scalar:
                            eng.activation(ot[:, :, c], hbs[c], Identity, bias=dap)
                        else:
                            eng.tensor_scalar(ot[:, :, c], hbs[c], dap, None, Add)
                    qs = queues[q % len(queues)]

                    def emit_store(tsub=tsub, ot=ot, qs=qs):
                        if isinstance(qs, tuple):
                            qa, qb = qs
                            qa.dma_start(out_d[b, ts(tsub, P), : L // 2],
                                         ot[:, : L // 2])
                            qb.dma_start(out_d[b, ts(tsub, P), L // 2 :],
                                         ot[:, L // 2 :])
                        else:
                            qs.dma_start(out_d[b, ts(tsub, P)], ot)

                    if ops_first:
                        stores.append(emit_store)
                    else:
                        emit_store()
                for st in stores:
                    st()

            # engine rotations: DVE is cheapest (2x SBUF mode); keep ACT free
            # for the Prelu/copy chain; Pool sparingly (SWDGE holds it)
            ENGS_STEADY = [nc.vector, nc.gpsimd, nc.vector, nc.vector]
            ENGS_TAIL = [nc.vector, nc.scalar]

            # --- schedule ----------------------------------------------------
            hts = hts0
            carry = None  # (b, lh_dep, hbs) pending half-1 emit
            for b in range(B_PER_CORE):
                last = b == B_PER_CORE - 1
                lh_head = {}
                if b == 0:
                    lh_head[0], lh_head[1] = mlp_mtile_pair_komajor(
                        "head", (0, 1), hts, KO_ARRIVAL)
                    lh_head[2] = mlp_mtile("head", 2, hts)
                    lh_head[3] = mlp_mtile("head", 3, hts)
                else:
                    lh_head[0] = mlp_mtile("head", 0, hts)
                    if carry is not None:
                        pb, p_lh_dep, p_hbs = carry
                        p_dep1 = dep_scores(p_lh_dep, 1)
                        out_tiles(pb, range(N_TSUB // 2, N_TSUB), p_dep1, p_hbs,
                                  ENGS_STEADY, [nc.gpsimd, nc.sync])
                        carry = None
                    for mt in range(1, N_MT):
                        lh_head[mt] = mlp_mtile("head", mt, hts)
                if b + 1 < B_PER_CORE:
                    hts_next = load_hidden(b + 1)
                if not last:
                    lh_dep = {}
                    lh_dep[0] = mlp_mtile("dep", 0, hts)
                    head_sb = head_scores(lh_head)
                    lh_dep[1] = mlp_mtile("dep", 1, hts)
                    lh_dep[2] = mlp_mtile("dep", 2, hts)
                    lh_dep[3] = mlp_mtile("dep", 3, hts)
                    hbs = [bcast(head_sb, 0), bcast(head_sb, 1)]
                    dp0 = dsc_open()
                    qs0 = [(q, q) for q in range(N_TSUB // 2)]
                    for mt in range(N_MT):
                        dsc_mt(dp0, lh_dep[mt], mt, qs0)
                    dep0 = dsc_close(dp0, N_TSUB // 2)
                    out_tiles(b, range(N_TSUB // 2), dep0, hbs,
                              ENGS_STEADY, [nc.sync, nc.gpsimd])
                    carry = (b, lh_dep, hbs)
                    hts = hts_next
                else:
                    # last batch: dep branch in (4, 2, 2)-tsub segments so
                    # earlier segments' stores flow while later segments
                    # still compute on the PE; only the final 2 tiles' store
                    # halves land after the last matmul
                    SEGS = [(0, 4), (4, 2), (6, 2)]
                    seg_q = [[nc.sync, nc.gpsimd],
                             [nc.sync, nc.gpsimd],
                             [(nc.scalar, nc.sync), (nc.gpsimd, nc.scalar)]]
                    seg_e = [ENGS_STEADY, [nc.vector], ENGS_TAIL]
                    # segment 0 interleaved with head scores/broadcast
                    t0, n0 = SEGS[0]
                    lh_s = {}
                    lh_s[0] = mlp_mtile_seg("dep", 0, hts, t0, n0)
                    head_sb = head_scores(lh_head)
                    lh_s[1] = mlp_mtile_seg("dep", 1, hts, t0, n0)
                    lh_s[2] = mlp_mtile_seg("dep", 2, hts, t0, n0)
                    lh_s[3] = mlp_mtile_seg("dep", 3, hts, t0, n0)
                    hbs = [bcast(head_sb, 0), bcast(head_sb, 1)]
                    dp = dsc_open()
                    qs = [(q, q) for q in range(n0)]
                    for mt in range(N_MT):
                        dsc_mt(dp, lh_s[mt], mt, qs)
                    dep_a = dsc_close(dp, n0)
                    out_tiles(b, range(t0, t0 + n0), dep_a, hbs,
                              seg_e[0], seg_q[0])
                    # segments 1 and 2 software-pipelined: seg1's last dsc
                    # group + outs hide behind seg2's first m-tile
                    t1, n1 = SEGS[1]
                    qs1 = [(q, q) for q in range(n1)]
                    lh1 = {}
                    lh1[0] = mlp_mtile_seg("dep", 0, hts, t1, n1)
                    lh1[1] = mlp_mtile_seg("dep", 1, hts, t1, n1)
                    dp1 = dsc_open()
                    dsc_mt(dp1, lh1[0], 0, qs1)
                    lh1[2] = mlp_mtile_seg("dep", 2, hts, t1, n1)
                    dsc_mt(dp1, lh1[1], 1, qs1)
                    lh1[3] = mlp_mtile_seg("dep", 3, hts, t1, n1)
                    dsc_mt(dp1, lh1[2], 2, qs1)
                    t2, n2 = SEGS[2]
                    qs2 = [(q, q) for q in range(n2)]
                    lh2 = {}
                    lh2[0] = mlp_mtile_seg("dep", 0, hts, t2, n2)
                    dsc_mt(dp1, lh1[3], 3, qs1)
                    dep1a = dsc_close(dp1, n1)
                    out_tiles(b, range(t1, t1 + n1), dep1a, hbs,
                              seg_e[1], seg_q[1])
                    lh2[1] = mlp_mtile_seg("dep", 1, hts, t2, n2)
                    dp2 = dsc_open()
                    dsc_mt(dp2, lh2[0], 0, qs2)
                    lh2[2] = mlp_mtile_seg("dep", 2, hts, t2, n2)
                    dsc_mt(dp2, lh2[1], 1, qs2)
                    lh2[3] = mlp_mtile_seg("dep", 3, hts, t2, n2)
                    dsc_mt(dp2, lh2[2], 2, qs2)
                    dsc_mt(dp2, lh2[3], 3, qs2)
                    dep2a = dsc_close(dp2, n2)
                    out_tiles(b, range(t2, t2 + n2), dep2a, hbs,
                              seg_e[2], seg_q[2], ops_first=True)

    nc.compile()
    return nc


def _prep_consts(W_dep, b_dep, W_head, b_head, Wc, bc):
    f = np.float32

    def arrange_w(W):
        Wp = np.zeros((D, MLP_PAD), f)
        Wp[:, :MLP] = W
        # [D, MLP_PAD] -> [P, N_MT, N_KO, P]: (ko p) (mt m) -> p mt ko m
        return (
            Wp.reshape(N_KO, P, N_MT, P).transpose(1, 2, 0, 3).copy()
        )

    def bias_t(bvec):
        bp = np.zeros((MLP_PAD,), f)
        bp[:MLP] = bvec
        bt = bp.reshape(N_MT, P).T  # [P, N_MT]
        out = np.empty((P, 2 * N_MT), f)
        out[:, :N_MT] = bt
        out[:, N_MT:] = -NEG_SLOPE * bt
        return out

    def arrange_wc(wc_half):
        wcp = np.zeros((MLP_PAD, 2), f)
        wcp[:MLP] = wc_half
        return wcp.reshape(N_MT, P, 2).transpose(1, 0, 2).copy()  # [P, mt, 2]

    return {
        "w_dep_r": arrange_w(W_dep),
        "w_head_r": arrange_w(W_head),
        "b_dep_t": bias_t(b_dep),
        "b_head_t": bias_t(b_head),
        "wc_dep_t": arrange_wc(Wc[:MLP]),
        "wc_head_t": arrange_wc(Wc[MLP:]),
        "sel_r": np.stack(
            [np.eye(2, dtype=f)[:, c : c + 1] @ np.ones((1, P), f) for c in range(2)],
            axis=1,
        ),
        "bc_2": bc.astype(f).reshape(2, 1).copy(),
    }


def kernel(hidden_state, W_dep, b_dep, W_head, b_head, Wc, bc):
    from concourse.bass_utils import run_bass_kernel_spmd

    hidden_state = np.asarray(hidden_state, dtype=np.float32)
    # host-side pre-transpose to [B, D, L] so the kernel needs no PE transposes
    hidden_t = np.ascontiguousarray(hidden_state.transpose(0, 2, 1))
    consts = _prep_consts(
        np.asarray(W_dep, np.float32),
        np.asarray(b_dep, np.float32),
        np.asarray(W_head, np.float32),
        np.asarray(b_head, np.float32),
        np.asarray(Wc, np.float32),
        np.asarray(bc, np.float32),
    )

    if "nc" not in _CACHE:
        _CACHE["nc"] = _build_nc()
    nc = _CACHE["nc"]

    in_maps = []
    for k in range(N_CORES):
        m = {"hidden_t": hidden_t[k * B_PER_CORE : (k + 1) * B_PER_CORE]}
        m.update(consts)
        in_maps.append(m)

    trace = bool(int(os.environ.get("BB_TRACE", "0")))
    if not trace:
        # The NTFF profiling hook (antenv.axon_hooks) is absent in this
        # container; a stray BASS_TRACE=1 would crash the run. Force off.
        os.environ["BASS_NEVER_TRACE"] = "1"
    res = run_bass_kernel_spmd(nc, in_maps, list(range(N_CORES)), trace=trace)
    _CACHE["last_results"] = res
    out = np.concatenate([res.results[k]["out"] for k in range(N_CORES)], axis=0)
    return out
